# revision 26
# baseline (speedup 1.0000x reference)
"""Trainium2 Bass kernel for nn_Decimate: 129-tap polyphase FIR decimation by q=4.

The reference's blocked-FFT conv is mathematically a strided valid correlation
    y[b, i] = sum_{j=0}^{128} x_ext[b, 4i + j] * k[j],   i in [0, 262144)
where x_ext = [reflect_64(x), x, zeros_64]  (length 1048704 = 128 * 8193).

Device scheme (per NeuronCore, 2 batch rows each across 8 cores):
  - x_ext is chunked into 128-element chunks, deinterleaved into 4 phase
    planes  plane_r[c', :] = chunk[4c' + r], split into bf16 hi + lo
    (pseudo-fp32), transposed to partition-major X[p, c'] and packed into
    one contiguous-per-partition slab tensor per (row, slab) — all on host,
    so the device does only large plain DMAs (1 MiB, 8.4 KiB/partition).
  - Toeplitz weights W_s[p, i0] = k[128 s + p - 4 i0] (5 shifts), split
    hi/lo.  W_s is nonzero only on an i0 band: s=0:[0,32) 1:[0,64)
    2:[32,96) 3:[64,128) 4:[96,128) — moving columns restricted to bands.
  - Tensor engine, signal stationary / weights moving:
        O[c', i0] = sum_s X_s[:, c'block].T @ W_s
    PSUM-accumulated over 15 banded matmuls (xh*wh + xh*wl + xl*wh).
  - O is produced [c', i0] so the store DMA is contiguous per partition.
"""

import numpy as np
import ml_dtypes

import concourse.bacc as bacc
import concourse.mybir as mybir
import concourse.tile as tile
from concourse.bass_utils import run_bass_kernel_spmd
from concourse.vector_clock import ScopedClock


class _LeanTile(tile.TileContext):
    """TileContext whose epilogue uses sem-only all-engine barriers.

    Keeps the full shutdown protocol (drain with global-clock waits, barrier,
    semaphore clears, barrier) so NEFF re-execution stays safe, but replaces
    the two drain-based multi_engine_barrier calls with the cheaper
    sem-inc/wait barrier flavor.
    """

    def _drain_and_barrier(self, tick_clock, wait_clock):
        drain_inst = self.nc.sync.drain()
        wait_clock.add_sem_waits(
            drain_inst.ins, ScopedClock({None: tick_clock.global_clock}))
        self.nc.all_engine_barrier(sem_only=True)
        popped = self.nc._tile_sem_poison_stack.pop()
        assert popped is self._sem_poison
        self.nc.clear_and_free_semaphores(
            list(self.sems.allocated().values()))
        self.nc.all_engine_barrier(sem_only=True)


bf16 = ml_dtypes.bfloat16

# Problem constants (hardcoded per harness contract)
T = 1048576
NTAP = 129
Q = 4
PAD = 64
ROWS = 16
N_CORES = 8
ROWS_PER_CORE = ROWS // N_CORES          # 2
OUT = T // Q                             # 262144 outputs per row
CBLK = 128                               # elements per input chunk
NCH_P = 8196                             # chunks, padded to multiple of 4
PLANE_COLS = NCH_P // 4                  # 2049
PLANE_ROWS = 2064                        # padded plane length
NCPRIME = OUT // CBLK                    # 2048 output chunks per row
SLAB_C = 512                             # output-chunk columns per slab
SLAB_W = 528                             # slab width incl. +1 halo
N_SLABS = NCPRIME // SLAB_C              # 4 slab groups per row
BLOCKS_PER_SLAB = SLAB_C // 128          # 4
NPLANE = 8                               # (hi/lo) x 4 phase planes

# i0-bands where W_s is nonzero
BAND = {0: (0, 32), 1: (0, 64), 2: (32, 96), 3: (64, 128), 4: (96, 128)}
# First combo: start=True zeroes the whole 2KB PSUM zero-region, so exactly
# one full-width start matmul (s=1; its zero columns initialize the rest),
# then banded accumulation.
COMBO1 = [
    (1, 0, 128, True), (0, 0, 32, False), (2, 32, 96, False),
    (3, 64, 128, False), (4, 96, 128, False),
]

_PROGRAM = None


def _build_weights(k):
    """W[s, p, i0] = k[128 s + p - 4 i0] masked to j in [0, 128]."""
    W = np.zeros((5, 128, 128), dtype=np.float32)
    p = np.arange(128)[:, None]
    i0 = np.arange(128)[None, :]
    for s in range(5):
        j = 128 * s + p - 4 * i0
        m = (j >= 0) & (j <= 128)
        W[s][m] = k[j[m]]
    return W


def _build_planes(x):
    """x: [B, T] fp32 -> phase planes [B, 4, PLANE_ROWS, 128] fp32."""
    B = x.shape[0]
    xe = np.zeros((B, NCH_P * CBLK), dtype=np.float32)
    xe[:, PAD:PAD + T] = x
    xe[:, :PAD] = x[:, 1:PAD + 1][:, ::-1]
    ch = xe.reshape(B, PLANE_COLS, 4, CBLK)
    planes = np.zeros((B, 4, PLANE_ROWS, CBLK), dtype=np.float32)
    planes[:, :, :PLANE_COLS, :] = ch.transpose(0, 2, 1, 3)
    return planes


def _build_program():
    """Build the per-core Bass/Tile program (same NEFF on all 8 cores)."""
    # Bacc (not raw Bass): its compile() splits multi-wait sync lists into
    # InstEventSemaphore chains — TRN2 allows only 1 wait per instruction.
    nc = bacc.Bacc(None)
    f32 = mybir.dt.float32
    b16 = mybir.dt.bfloat16

    # xs[row, slab, p, (plane, c)] — per-partition contiguous 8448 B
    xs = nc.declare_dram_parameter(
        "xs", [ROWS_PER_CORE, N_SLABS, CBLK, NPLANE * SLAB_W], b16,
        isOutput=False)
    # w[p, (combo plane), i0]: 0..4 = wh_s, 5..9 = wl_s
    w = nc.declare_dram_parameter("w", [CBLK, 10, CBLK], b16, isOutput=False)
    y = nc.declare_dram_parameter(
        "y", [ROWS_PER_CORE, NCPRIME, CBLK], f32, isOutput=True)

    with _LeanTile(nc) as tc:
        with (
            tc.tile_pool(name="wpool", bufs=1) as wpool,
            tc.tile_pool(name="xpool", bufs=4) as xpool,
            tc.tile_pool(name="opool", bufs=3) as opool,
            tc.tile_pool(name="psum", bufs=8, space="PSUM") as psum_pool,
        ):
            w_t = wpool.tile([CBLK, 10, CBLK], b16, tag="w")
            nc.scalar.dma_start(out=w_t[:], in_=w[:])

            def xsl(t, hl, r, c0):
                """Stationary slice: plane (hl, r), local cols [c0, c0+128)."""
                base = (2 * r + hl) * SLAB_W + c0
                return t[:, base:base + 128]

            H = NPLANE * SLAB_W // 2
            for row in range(ROWS_PER_CORE):
                for g in range(N_SLABS):
                    t = xpool.tile([CBLK, NPLANE * SLAB_W], b16, tag="xs")
                    # split the slab load in half for a faster pipeline ramp
                    nc.sync.dma_start(out=t[:, :H], in_=xs[row, g, :, :H])
                    nc.sync.dma_start(out=t[:, H:], in_=xs[row, g, :, H:])
                    stage = opool.tile([CBLK, BLOCKS_PER_SLAB, CBLK], f32,
                                       tag="stage")
                    for bl in range(BLOCKS_PER_SLAB):
                        O = psum_pool.tile([CBLK, CBLK], f32, tag="O")
                        nmm = len(COMBO1) + 10
                        i = 0
                        # combo 1: xh * wh, split bands, first-touch starts
                        for s, lo, hi, st in COMBO1:
                            r, off = s % 4, s // 4
                            i += 1
                            nc.tensor.matmul(
                                O[:, lo:hi],
                                xsl(t, 0, r, 128 * bl + off),
                                w_t[:, s, lo:hi],
                                start=st, stop=False)
                        # combo 2: xh * wl;  combo 3: xl * wh
                        for hl, wofs in ((0, 5), (1, 0)):
                            for s in range(5):
                                r, off = s % 4, s // 4
                                lo, hi = BAND[s]
                                i += 1
                                nc.tensor.matmul(
                                    O[:, lo:hi],
                                    xsl(t, hl, r, 128 * bl + off),
                                    w_t[:, wofs + s, lo:hi],
                                    start=False, stop=(i == nmm))
                        nc.vector.tensor_copy(stage[:, bl, :], O[:])
                    c_base = SLAB_C * g
                    # y[row, c_base + 128*bl + c', i] <- stage[c', bl, i]
                    dst = y[row, c_base:c_base + SLAB_C, :].rearrange(
                        "(b c) i -> c b i", b=BLOCKS_PER_SLAB)
                    nc.scalar.dma_start(out=dst, in_=stage[:])
    nc.finalize()
    return nc


def _get_program():
    global _PROGRAM
    if _PROGRAM is None:
        _PROGRAM = _build_program()
    return _PROGRAM


def _prepare_in_maps(x, k):
    planes = _build_planes(np.ascontiguousarray(x, dtype=np.float32))
    ph = planes.astype(bf16)
    pl = (planes - ph.astype(np.float32)).astype(bf16)
    # host-side transpose to partition-major [B, 4, p, c]
    ph = np.ascontiguousarray(ph.swapaxes(2, 3))
    pl = np.ascontiguousarray(pl.swapaxes(2, 3))

    # pack [B, slab, p, (plane, c_local)] with per-partition contiguity
    B = x.shape[0]
    xsv = np.zeros((B, N_SLABS, CBLK, NPLANE, SLAB_W), dtype=bf16)
    for g in range(N_SLABS):
        sl = slice(SLAB_C * g, SLAB_C * g + SLAB_W)
        for r in range(4):
            xsv[:, g, :, 2 * r + 0, :] = ph[:, r, :, sl]
            xsv[:, g, :, 2 * r + 1, :] = pl[:, r, :, sl]
    xsv = xsv.reshape(B, N_SLABS, CBLK, NPLANE * SLAB_W)

    W = _build_weights(np.asarray(k, dtype=np.float32))
    wh = W.astype(bf16)
    wl = (W - wh.astype(np.float32)).astype(bf16)
    # weight layout [p, (wh 0..4 | wl 0..4), i0]
    w_t = np.concatenate(
        [np.transpose(wh, (1, 0, 2)), np.transpose(wl, (1, 0, 2))], axis=1)
    w_t = np.ascontiguousarray(w_t)

    in_maps = []
    for c in range(N_CORES):
        sl = slice(c * ROWS_PER_CORE, (c + 1) * ROWS_PER_CORE)
        in_maps.append({
            "xs": np.ascontiguousarray(xsv[sl]),
            "w": w_t,
        })
    return in_maps


def _run(x, k, trace=False):
    nc = _get_program()
    in_maps = _prepare_in_maps(x, k)
    res = run_bass_kernel_spmd(nc, in_maps, list(range(N_CORES)), trace=trace)
    outs = [np.asarray(r["y"], dtype=np.float32) for r in res.results]
    out = np.concatenate(outs, axis=0).reshape(ROWS, OUT)
    return out, res


def kernel(x, kernel, q):
    assert int(q) == Q and x.shape == (ROWS, T) and kernel.shape == (NTAP,)
    out, _ = _run(np.asarray(x), np.asarray(kernel), trace=False)
    return out


def kernel_traced(x, kernel, q):
    """Like kernel() but returns (out, BassKernelResults) with HW profile."""
    out, res = _run(np.asarray(x), np.asarray(kernel), trace=True)
    return out, res
